# revision 1
# baseline (speedup 1.0000x reference)
"""Trainium2 Bass kernel for nn_EntityEncoder (multi-hot embedding bag + MLP head).

Strategy: vocab (E) sharding across 8 cores. The host lays out each core's
[512, 6250] int32 mask slice in transposed, zero-padded, SBUF-ready form
[128, 50*512] (partition = e-within-subtile, free = subtile-major bp), and the
matching embedding rows as [128, 50*128]. Each core:
  - DMAs its ~13 MB mask slice in 6 large 128-partition transfers (near peak
    HBM bandwidth; non-128-partition DMAs run 2.4x slower) and the 3.3 MB
    embedding slice in 10 pipelined chunks,
  - casts mask int32 -> bf16 on DVE, embedding f32 -> bf16 on ACT,
  - accumulates sums[h, bp] over 50 K=128 subtiles as bf16 matmuls straight
    from the resident tiles (no on-chip transposes),
  - computes counts with exact bf16 pair/quad sums on DVE plus a few
    ones-matmuls,
  - exchanges partials with a single AllToAll (cheapest collective) and
    reduces the 8 contributions locally on DVE,
  - computes the head (divide, path-mean, LN -> Linear+ReLU -> BN, x2) for
    its own 4 batches only; the host concatenates the 8 [4, 128] outputs.
LN gamma/beta are folded into the following linear's weights on the host.
"""

import numpy as np

B, P, E, H = 32, 16, 50000, 128
NCORES = 8
BP = B * P                 # 512
E_SH = E // NCORES         # 6250 vocab rows per core
SUB = 128                  # matmul K subtile (shard zero-padded to 6400)
E_PAD = 6400               # padded vocab rows per core
NSUB = E_PAD // SUB        # 50
TILE_SPLIT = [2, 12, 12, 12, 8, 4]  # subtiles per tile (small fill + small tail)
EPS = 1e-5
NB = BP // NCORES          # 64 paths per core after AllToAll
BL = B // NCORES           # 4 local batches

# packed params layout: [128, NPAR] f32
#  col 4 bn1_g', 5 bn1_b, 6 bn2_g', 7 bn2_b, 8 b1', 9 b2',
#  10:138 (w1*ln1_g)^T, 138:266 (w2*ln2_g)^T
NPAR = 266

_cached = {}


def _build():
    import concourse.bacc as bacc
    import concourse.mybir as mybir
    import concourse.tile as tile
    from concourse import masks

    f32 = mybir.dt.float32
    bf16 = mybir.dt.bfloat16
    i32 = mybir.dt.int32

    nc = bacc.Bacc("TRN2", target_bir_lowering=False, debug=False,
                   num_devices=NCORES)

    x_d = nc.dram_tensor("x", [SUB, NSUB * BP], i32, kind="ExternalInput")
    emb_d = nc.dram_tensor("emb", [SUB, NSUB * H], f32, kind="ExternalInput")
    par_d = nc.dram_tensor("par", [128, NPAR], f32, kind="ExternalInput")
    out_d = nc.dram_tensor("out", [BL, H], f32, kind="ExternalOutput")

    with tile.TileContext(nc) as tc:
        with tc.tile_pool(name="const", bufs=1) as constp, \
             tc.tile_pool(name="xin", bufs=3) as xin, \
             tc.tile_pool(name="xfp", bufs=3) as xfp, \
             tc.tile_pool(name="cntp", bufs=2) as cntp, \
             tc.tile_pool(name="head", bufs=1) as head, \
             tc.tile_pool(name="ps_acc", bufs=1, space="PSUM") as ps_acc, \
             tc.tile_pool(name="ps_misc", bufs=3, space="PSUM") as ps_misc, \
             tc.tile_pool(name="dram", bufs=1, space="DRAM") as dram:

            ident = constp.tile([128, 128], f32)
            masks.make_identity(nc, ident[:])
            ones_col = constp.tile([128, 1], f32)
            nc.vector.memset(ones_col[:], 1.0)
            ones_row = constp.tile([1, 128], f32)
            nc.vector.memset(ones_row[:], 1.0)
            zero_1 = constp.tile([1, 1], f32)
            nc.vector.memset(zero_1[:], 0.0)
            ones_bf = constp.tile([128, 1], bf16)
            nc.vector.memset(ones_bf[:], 1.0)

            # resident embedding: load f32, cast to bf16 on ACT in chunks
            emb_f = constp.tile([SUB, NSUB * H], f32)
            emb_b = constp.tile([SUB, NSUB * H], bf16)
            NEC = 10
            EC = NSUB * H // NEC
            for k in range(NEC):
                nc.scalar.dma_start(emb_f[:, k * EC:(k + 1) * EC],
                                    emb_d[:, k * EC:(k + 1) * EC])
                nc.scalar.copy(emb_b[:, k * EC:(k + 1) * EC],
                               emb_f[:, k * EC:(k + 1) * EC])

            par = constp.tile([128, NPAR], f32)
            nc.scalar.dma_start(par[:], par_d[:, :])

            # preload the Sqrt ACT table so the head doesn't pay the
            # 1.3us table load on the critical path
            warm = constp.tile([1, 1], f32)
            nc.scalar.activation(warm[:], zero_1[:],
                                 mybir.ActivationFunctionType.Sqrt,
                                 bias=zero_1[:, :1], scale=1.0)

            # tiny warm-up AllToAll: pays the first-collective ncfw init +
            # cross-rank sync early, overlapped with the input DMAs.
            # high_priority places it at the very start of each engine stream.
            with tc.high_priority():
                zrow = constp.tile([1, 128], bf16)
                nc.vector.memset(zrow[:], 0.0)
                ccw_in = dram.tile([NCORES * 2, 8], bf16)
                ccw_out = dram.tile([NCORES * 2, 8], bf16)
                nc.scalar.dma_start(
                    ccw_in[:, :].rearrange("(s r) c -> s (r c)", r=2),
                    zrow[:].rearrange("p (s c) -> p s c", c=16))
                nc.gpsimd.collective_compute(
                    "AllToAll",
                    mybir.AluOpType.bypass,
                    replica_groups=[list(range(NCORES))],
                    ins=[ccw_in[:].opt()],
                    outs=[ccw_out[:].opt()],
                )

            # ---------------- main GEMM loop ----------------
            psum_sums = ps_acc.tile([128, BP], f32)   # [h, bp]
            psum_cnt = ps_acc.tile([1, BP], f32)

            # count group bounds: greedy groups of <=4 subtiles per tile
            n_cnt_groups = sum((s + 3) // 4 for s in TILE_SPLIT)
            cnt_done = 0
            sidx = 0
            for t, S in enumerate(TILE_SPLIT):
                XWT = S * BP
                xi = xin.tile([SUB, XWT], i32, tag=f"xi{min(t,1)}",
                              name=f"xi{t}")
                nc.sync.dma_start(
                    xi[:], x_d[:, sidx * BP:(sidx + S) * BP])
                xf = xfp.tile([SUB, XWT], bf16, tag=f"xf{min(t,1)}",
                              name=f"xf{t}")
                nc.vector.tensor_copy(xf[:], xi[:])
                for j in range(S):
                    g = sidx + j
                    nc.tensor.matmul(
                        psum_sums[:],
                        emb_b[:, g * H:(g + 1) * H],
                        xf[:, j * BP:(j + 1) * BP],
                        start=(g == 0), stop=(g == NSUB - 1))
                # counts: combine subtiles in groups of <=4 with exact bf16
                # adds on GpSimd, then one ones-matmul per group
                j = 0
                while j < S:
                    gs = min(4, S - j)
                    if gs >= 2:
                        p0 = cntp.tile([SUB, BP], bf16, tag="pr", name="p0")
                        nc.vector.tensor_tensor(
                            out=p0[:], in0=xf[:, j * BP:(j + 1) * BP],
                            in1=xf[:, (j + 1) * BP:(j + 2) * BP],
                            op=mybir.AluOpType.add)
                        grp = p0
                    else:
                        grp = None  # single subtile
                    if gs == 4:
                        p1 = cntp.tile([SUB, BP], bf16, tag="pr", name="p1")
                        nc.vector.tensor_tensor(
                            out=p1[:], in0=xf[:, (j + 2) * BP:(j + 3) * BP],
                            in1=xf[:, (j + 3) * BP:(j + 4) * BP],
                            op=mybir.AluOpType.add)
                        q = cntp.tile([SUB, BP], bf16, tag="q", name="q")
                        nc.vector.tensor_tensor(
                            out=q[:], in0=p0[:], in1=p1[:],
                            op=mybir.AluOpType.add)
                        grp = q
                    elif gs == 3:
                        q = cntp.tile([SUB, BP], bf16, tag="q", name="q")
                        nc.vector.tensor_tensor(
                            out=q[:], in0=p0[:],
                            in1=xf[:, (j + 2) * BP:(j + 3) * BP],
                            op=mybir.AluOpType.add)
                        grp = q
                    rhs = grp[:] if grp is not None \
                        else xf[:, j * BP:(j + 1) * BP]
                    nc.tensor.matmul(
                        psum_cnt[:], ones_bf[:, :], rhs,
                        start=(cnt_done == 0),
                        stop=(cnt_done == n_cnt_groups - 1))
                    cnt_done += 1
                    j += gs
                sidx += S

            # ------------- AllToAll exchange (bf16 + raw f32 cnt) -----
            # 8 blocks of 130 rows: 128 bf16 sums rows + 2 rows carrying the
            # f32 counts as raw bytes (AllToAll is bypass; no arithmetic).
            cc_in = dram.tile([NCORES * 130, NB], bf16)
            cc_out = dram.tile([NCORES * 130, NB], bf16)
            sums_stage = head.tile([128, BP], bf16)
            nc.any.tensor_copy(sums_stage[:], psum_sums[:])
            cnt_stage = head.tile([1, BP], f32)
            nc.vector.tensor_copy(cnt_stage[:], psum_cnt[:])
            cc_in_v = cc_in[:].rearrange("(s r) c -> r s c", r=130)
            nc.scalar.dma_start(
                cc_in_v[0:128],
                sums_stage[:].rearrange("p (s c) -> p s c", c=NB))
            nc.scalar.dma_start(
                cc_in_v[128:130].rearrange("r s c -> s (r c)"),
                cnt_stage[:].bitcast(bf16).rearrange(
                    "p (s c) -> p s c", c=2 * NB))
            nc.gpsimd.collective_compute(
                "AllToAll",
                mybir.AluOpType.bypass,
                replica_groups=[list(range(NCORES))],
                ins=[cc_in[:].opt()],
                outs=[cc_out[:].opt()],
            )
            sums8 = head.tile([128, BP], bf16)
            cnt8w = head.tile([1, 2 * BP], bf16)
            cc_out_v = cc_out[:].rearrange("(s r) c -> r s c", r=130)
            nc.scalar.dma_start(
                sums8[:].rearrange("p (s c) -> p s c", c=NB),
                cc_out_v[0:128])
            nc.scalar.dma_start(
                cnt8w[:].rearrange("p (s c) -> p s c", c=2 * NB),
                cc_out_v[128:130].rearrange("r s c -> s (r c)"))
            cnt8 = cnt8w[:].bitcast(f32)

            # local reduce of the 8 contributions: fold the free dim in
            # halves (chunk pairing is arbitrary for a sum) - 3 wide adds
            def tree_reduce(src, parts, width, name, eng=None):
                cur = src
                w = NCORES * width
                lev = 0
                while w > width:
                    h = w // 2
                    o = head.tile([parts, h], f32, bufs=1,
                                  tag=f"{name}r{lev}", name=f"{name}_{lev}")
                    (eng or nc.vector).tensor_tensor(
                        out=o[:], in0=cur[:, 0:h], in1=cur[:, h:w],
                        op=mybir.AluOpType.add)
                    cur = o
                    w = h
                    lev += 1
                return cur[:, 0:width]

            sums_loc = tree_reduce(sums8[:], 128, NB, "s")
            cnt_loc = tree_reduce(cnt8, 1, NB, "c", eng=nc.gpsimd)

            # ---------------- head (local 64 paths / 4 batches) -------
            bc_ps = ps_misc.tile([128, NB], f32, tag="psmisc")
            nc.tensor.matmul(bc_ps[:], ones_row[:], cnt_loc,
                             start=True, stop=True)
            rec = head.tile([128, NB], f32)
            nc.vector.reciprocal(rec[:], bc_ps[:])
            path = head.tile([128, NB], f32)
            nc.vector.tensor_tensor(out=path[:], in0=sums_loc,
                                    in1=rec[:], op=mybir.AluOpType.mult)
            x0 = head.tile([128, BL], f32)
            nc.vector.reduce_sum(
                x0[:], path[:].rearrange("h (b p) -> h b p", p=P),
                axis=mybir.AxisListType.X)

            def layer_norm(x_sb, eps_val, name):
                xx = head.tile([128, 2 * BL], f32, tag=f"{name}_xx")
                nc.vector.tensor_copy(xx[:, 0:BL], x_sb[:])
                nc.vector.tensor_tensor(out=xx[:, BL:2 * BL], in0=x_sb[:],
                                        in1=x_sb[:], op=mybir.AluOpType.mult)
                st_ps = ps_misc.tile([1, 2 * BL], f32, tag="psmisc")
                nc.tensor.matmul(st_ps[:], ones_col[:], xx[:],
                                 start=True, stop=True)
                # mu = Sx/128 ; var+eps = (Sx2/128 + eps) - mu^2
                mr = head.tile([1, 2 * BL], f32, tag=f"{name}_mr")
                nc.vector.tensor_scalar(
                    out=mr[:, 0:BL], in0=st_ps[:, 0:BL],
                    scalar1=1.0 / 128, scalar2=None,
                    op0=mybir.AluOpType.mult)
                mu2 = head.tile([1, BL], f32, tag=f"{name}_mu2")
                nc.vector.tensor_tensor(
                    out=mu2[:], in0=mr[:, 0:BL], in1=mr[:, 0:BL],
                    op=mybir.AluOpType.mult)
                var = head.tile([1, BL], f32, tag=f"{name}_var")
                nc.vector.tensor_scalar(
                    out=var[:], in0=st_ps[:, BL:2 * BL],
                    scalar1=1.0 / 128, scalar2=float(eps_val),
                    op0=mybir.AluOpType.mult, op1=mybir.AluOpType.add)
                nc.vector.tensor_tensor(
                    out=var[:], in0=var[:], in1=mu2[:],
                    op=mybir.AluOpType.subtract)
                sd = head.tile([1, BL], f32, tag=f"{name}_sd")
                nc.scalar.activation(sd[:], var[:],
                                     mybir.ActivationFunctionType.Sqrt,
                                     bias=zero_1[:, :1], scale=1.0)
                nc.vector.reciprocal(mr[:, BL:2 * BL], sd[:])
                bcs = ps_misc.tile([128, 2 * BL], f32, tag="psmisc")
                nc.tensor.matmul(bcs[:], ones_row[:], mr[:],
                                 start=True, stop=True)
                xn = head.tile([128, BL], f32, tag=f"{name}_xn")
                nc.vector.tensor_tensor(
                    out=xn[:], in0=x_sb[:], in1=bcs[:, 0:BL],
                    op=mybir.AluOpType.subtract)
                nc.vector.tensor_tensor(
                    out=xn[:], in0=xn[:], in1=bcs[:, BL:2 * BL],
                    op=mybir.AluOpType.mult)
                return xn

            def linear_relu_bn(x_sb, w_lo, b_col, bng_col, bnb_col, name):
                y_ps = ps_misc.tile([128, BL], f32, tag="psmisc")
                nc.tensor.matmul(y_ps[:], par[:, w_lo:w_lo + 128], x_sb[:],
                                 start=True, stop=True)
                y = head.tile([128, BL], f32, tag=f"{name}_relu")
                nc.vector.tensor_scalar(
                    out=y[:], in0=y_ps[:],
                    scalar1=par[:, b_col:b_col + 1], scalar2=0.0,
                    op0=mybir.AluOpType.add, op1=mybir.AluOpType.max)
                z = head.tile([128, BL], f32, tag=f"{name}_bn")
                nc.vector.tensor_scalar(
                    out=z[:], in0=y[:],
                    scalar1=par[:, bng_col:bng_col + 1],
                    scalar2=par[:, bnb_col:bnb_col + 1],
                    op0=mybir.AluOpType.mult, op1=mybir.AluOpType.add)
                return z

            # LN1 on un-normalized p-sum: eps scales by P^2
            h1 = layer_norm(x0, EPS * P * P, "ln1")
            h2 = linear_relu_bn(h1, 10, 8, 4, 5, "l1")
            h3 = layer_norm(h2, EPS, "ln2")
            h4 = linear_relu_bn(h3, 138, 9, 6, 7, "l2")

            # transpose [128h, 4b] -> [4b, 128h] and store
            out_ps = ps_misc.tile([BL, 128], f32, tag="psmisc")
            nc.tensor.transpose(out_ps[:], h4[:], ident[:, :])
            out_sb = head.tile([BL, 128], f32)
            nc.vector.tensor_copy(out_sb[:], out_ps[:])
            nc.scalar.dma_start(out_d[:, :], out_sb[:])

    nc.compile()
    return nc


def _prepare_in_maps(inputs):
    x = np.asarray(inputs["inputs"])
    emb = np.asarray(inputs["emb"], dtype=np.float32)
    w1 = np.asarray(inputs["w1"], dtype=np.float32)
    b1 = np.asarray(inputs["b1"], dtype=np.float32)
    w2 = np.asarray(inputs["w2"], dtype=np.float32)
    b2 = np.asarray(inputs["b2"], dtype=np.float32)

    par = np.zeros((128, NPAR), dtype=np.float32)
    par[:, 4] = np.asarray(inputs["bn1_g"], np.float32) / np.sqrt(
        np.float32(1.0) + np.float32(EPS))
    par[:, 5] = inputs["bn1_b"]
    par[:, 6] = np.asarray(inputs["bn2_g"], np.float32) / np.sqrt(
        np.float32(1.0) + np.float32(EPS))
    par[:, 7] = inputs["bn2_b"]
    ln1_g = np.asarray(inputs["ln1_g"], np.float32)
    ln1_b = np.asarray(inputs["ln1_b"], np.float32)
    ln2_g = np.asarray(inputs["ln2_g"], np.float32)
    ln2_b = np.asarray(inputs["ln2_b"], np.float32)
    # y = W @ (g*xn + b) + b1 = (W*g) @ xn + (W@b + b1)
    w1f = w1 * ln1_g[None, :]
    b1f = b1 + w1 @ ln1_b
    w2f = w2 * ln2_g[None, :]
    b2f = b2 + w2 @ ln2_b
    par[:, 8] = b1f
    par[:, 9] = b2f
    par[:, 10:138] = w1f.T
    par[:, 138:266] = w2f.T

    x_flat = x.reshape(BP, E)
    in_maps = []
    for c in range(NCORES):
        lo = c * E_SH
        # [bp, e] slice -> pad e to 6400 -> [p, j, bp] -> [128, 50*512]
        seg_t = np.zeros((E_PAD, BP), dtype=np.int32)
        seg_t[:E_SH] = x_flat[:, lo:lo + E_SH].T
        x_sh = np.ascontiguousarray(
            seg_t.reshape(NSUB, SUB, BP).transpose(1, 0, 2)
        ).reshape(SUB, NSUB * BP)
        # emb rows -> pad -> [p, j, h] -> [128, 50*128]
        seg_e = np.zeros((E_PAD, H), dtype=np.float32)
        seg_e[:E_SH] = emb[lo:lo + E_SH, :]
        if c == 0:
            seg_e[0, :] = 0.0   # padding_idx=0
        emb_sh = np.ascontiguousarray(
            seg_e.reshape(NSUB, SUB, H).transpose(1, 0, 2)
        ).reshape(SUB, NSUB * H)
        in_maps.append({"x": x_sh, "emb": emb_sh, "par": par})
    return in_maps


def _run(inputs, trace=False):
    from concourse.bass_utils import run_bass_kernel_spmd

    if "nc" not in _cached:
        _cached["nc"] = _build()
    nc = _cached["nc"]
    in_maps = _prepare_in_maps(inputs)
    res = run_bass_kernel_spmd(
        nc, in_maps, core_ids=list(range(NCORES)), trace=trace)
    out = np.concatenate(
        [np.asarray(res.results[c]["out"]) for c in range(NCORES)], axis=0)
    return out, res.exec_time_ns


def kernel(**inputs) -> np.ndarray:
    out, _ = _run(inputs, trace=False)
    return out



# revision 3
# speedup vs baseline: 1.0779x; 1.0779x over previous
"""Trainium2 Bass kernel for nn_EntityEncoder (multi-hot embedding bag + MLP head).

Strategy: vocab (E) sharding across 8 cores.

Host prep per core:
  - x slice [512, 6250] -> transposed/padded [128, 50*512] fp8_e4m3 (values
    0/1 exact; 3.28 MB instead of 13.1 MB int32),
  - emb shard + ones column [128, 50*129] bf16 (1.66 MB) -- the ones column
    makes the per-path COUNT fall out of the same matmuls as column 128,
  - params with LN gamma/beta folded into the linear weights.

Device per core:
  - main GEMM oriented out[bp, 129]: for each of 50 K=128 subtiles and each
    of 4 bp-quarters, matmul(psum_q, lhsT=x_tile, rhs=[emb|1]) accumulates
    sums AND counts in 4 PSUM banks. fp8 lhsT x bf16 rhs, fp32 accum.
  - one ReduceScatter(add) over [512, 129] f32 partials: each core receives
    the fully-summed [64, 129] for its own 64 paths (its 4 batches).
  - head in [b,h]<->[h,b] alternating layout: divide-by-count and path-mean
    fused into one tiny matmul via a rec-scaled block-mask; LN stats via
    bn_stats/bn_aggr; bias+relu and bn as per-partition tensor_scalar ops.
  - a tiny warmup ReduceScatter issued at priority 0 absorbs the ncfw
    init/barrier cost under the main loop.
"""

import numpy as np

B, P, E, H = 32, 16, 50000, 128
NCORES = 8
BP = B * P                  # 512
E_SH = E // NCORES          # 6250
SUB = 128
E_PAD = 6400
NSUB = E_PAD // SUB         # 50
NCH = 5                     # x DMA chunks
SPC = NSUB // NCH           # 10 subtiles per chunk
EPS = 1e-5
NB = BP // NCORES           # 64 local paths
BL = B // NCORES            # 4 local batches

# packed params [128, NPAR] f32:
# col 0 b1f, 1 b2f, 2 bn1_g', 3 bn1_b, 4 bn2_g', 5 bn2_b,
# col 6 eps1 (EPS*P*P), 7 eps2 (EPS), cols 8:12 M block-mask (rows 0:64),
# cols 12:140 (w1*ln1_g)^T, cols 140:268 (w2*ln2_g)^T
NPAR = 268

_cached = {}


def _build():
    import concourse.bacc as bacc
    import concourse.mybir as mybir
    import concourse.tile as tile
    from concourse import masks

    f32 = mybir.dt.float32
    bf16 = mybir.dt.bfloat16
    fp8 = mybir.dt.float8e4

    nc = bacc.Bacc("TRN2", target_bir_lowering=False, debug=False,
                   num_devices=NCORES)

    x_d = nc.dram_tensor("x", [SUB, NSUB * BP], fp8, kind="ExternalInput")
    emb_d = nc.dram_tensor("emb", [SUB, NSUB * 129], bf16,
                           kind="ExternalInput")
    par_d = nc.dram_tensor("par", [128, NPAR], f32, kind="ExternalInput")
    out_d = nc.dram_tensor("out", [BL, H], f32, kind="ExternalOutput")

    groups = [list(range(NCORES))]

    with tile.TileContext(nc) as tc:
        with tc.tile_pool(name="const", bufs=1) as constp, \
             tc.tile_pool(name="xin", bufs=3) as xin, \
             tc.tile_pool(name="head", bufs=1) as head, \
             tc.tile_pool(name="ps_acc", bufs=1, space="PSUM") as ps_acc, \
             tc.tile_pool(name="ps_head", bufs=4, space="PSUM") as ps_head, \
             tc.tile_pool(name="dram", bufs=1) as dram:

            # ---- warmup collective: pay ncfw init early, overlapped ----
            with tc.high_priority():
                wz = constp.tile([8, 8], f32)
                nc.vector.memset(wz[:], 0.0)
                ccw_in = dram.tile([8, 8], f32, space="DRAM")
                ccw_out = dram.tile([1, 8], f32, space="DRAM")
                nc.scalar.dma_start(ccw_in[:], wz[:])
                nc.gpsimd.collective_compute(
                    "ReduceScatter",
                    mybir.AluOpType.add,
                    replica_groups=groups,
                    ins=[ccw_in[:].opt()],
                    outs=[ccw_out[:].opt()],
                )

            # ---- constants ----
            ident = constp.tile([128, 128], f32)
            masks.make_identity(nc, ident[:])
            par = constp.tile([128, NPAR], f32)
            nc.scalar.dma_start(par[:], par_d[:, :])

            # resident [emb | ones] bf16, 2 chunks
            emb_b = constp.tile([SUB, NSUB * 129], bf16)
            EC = NSUB * 129 // 2
            for k in range(2):
                nc.scalar.dma_start(emb_b[:, k * EC:(k + 1) * EC],
                                    emb_d[:, k * EC:(k + 1) * EC])

            # preload Sqrt ACT table off the critical path
            warm = constp.tile([1, 1], f32)
            nc.scalar.activation(warm[:], par[0:1, 7:8],
                                 mybir.ActivationFunctionType.Sqrt,
                                 bias=par[0:1, 7:8], scale=1.0)

            # ---- main GEMM: out[bp, 129] in 4 PSUM banks ----
            ps = [ps_acc.tile([128, 512], f32, name=f"acc{q}")
                  for q in range(4)]
            for t in range(NCH):
                xt = xin.tile([SUB, SPC * BP], fp8, tag="xt", name=f"xt{t}")
                nc.sync.dma_start(
                    xt[:], x_d[:, t * SPC * BP:(t + 1) * SPC * BP])
                for j in range(SPC):
                    g = t * SPC + j
                    rhs = emb_b[:, g * 129:(g + 1) * 129]
                    for q in range(4):
                        nc.tensor.matmul(
                            ps[q][:, 0:129],
                            xt[:, j * BP + q * 128: j * BP + (q + 1) * 128],
                            rhs,
                            start=(g == 0), stop=(g == NSUB - 1))

            # ---- stage partials + ReduceScatter ----
            stage = head.tile([128, 4 * 129], f32)
            for q in range(4):
                if q % 2 == 0:
                    nc.vector.tensor_copy(stage[:, q * 129:(q + 1) * 129],
                                          ps[q][:, 0:129])
                else:
                    nc.scalar.copy(stage[:, q * 129:(q + 1) * 129],
                                   ps[q][:, 0:129])
            cc_in = dram.tile([BP, 129], f32, space="DRAM")
            cc_out = dram.tile([NB, 129], f32, space="DRAM")
            nc.sync.dma_start(
                cc_in[:].rearrange("(q p) c -> p q c", p=128),
                stage[:].rearrange("p (q c) -> p q c", c=129))
            nc.gpsimd.collective_compute(
                "ReduceScatter",
                mybir.AluOpType.add,
                replica_groups=groups,
                ins=[cc_in[:].opt()],
                outs=[cc_out[:].opt()],
            )

            # ---- head on [64, 129] totals ----
            S = head.tile([NB, 129], f32)
            nc.sync.dma_start(S[:], cc_out[:])

            rec = head.tile([NB, 1], f32)
            nc.vector.reciprocal(rec[:], S[:, 128:129])
            R = head.tile([NB, BL], f32)
            nc.vector.tensor_scalar(
                out=R[:], in0=par[0:NB, 8:12], scalar1=rec[:, 0:1],
                scalar2=None, op0=mybir.AluOpType.mult)

            # x0[b, h] = sum_p sums[p, h] / cnt[p]   (= P * mean; LN-invariant)
            x0_ps = ps_head.tile([BL, 128], f32, tag="psh", name="x0")
            nc.tensor.matmul(x0_ps[:], R[:], S[:, 0:128],
                             start=True, stop=True)

            def layer_norm(x_ps, eps_col, name):
                # x_ps: [BL, 128] PSUM -> xn [BL, 128] SBUF
                st6 = head.tile([BL, 6], f32, tag=f"{name}_st6")
                nc.vector.bn_stats(st6[:], x_ps[:])
                mv = head.tile([BL, 2], f32, tag=f"{name}_mv")
                nc.vector.bn_aggr(mv[:], st6[:])
                sd = head.tile([BL, 1], f32, tag=f"{name}_sd")
                nc.scalar.activation(sd[:], mv[:, 1:2],
                                     mybir.ActivationFunctionType.Sqrt,
                                     bias=par[0:BL, eps_col:eps_col + 1],
                                     scale=1.0)
                rstd = head.tile([BL, 1], f32, tag=f"{name}_rstd")
                nc.vector.reciprocal(rstd[:], sd[:])
                xn = head.tile([BL, 128], f32, tag=f"{name}_xn")
                nc.vector.tensor_scalar(
                    out=xn[:], in0=x_ps[:],
                    scalar1=mv[:, 0:1], scalar2=rstd[:, 0:1],
                    op0=mybir.AluOpType.subtract, op1=mybir.AluOpType.mult)
                return xn

            def linear_relu_bn(xn, w_lo, b_col, bng_col, bnb_col, name):
                # xn [BL, 128] -> z [128, BL]
                xt_ps = ps_head.tile([128, BL], f32, tag="psh",
                                     name=f"{name}_xt")
                nc.tensor.transpose(xt_ps[:], xn[:], ident[0:BL, 0:BL])
                xt_sb = head.tile([128, BL], f32, tag=f"{name}_xts")
                nc.vector.tensor_copy(xt_sb[:], xt_ps[:])
                y_ps = ps_head.tile([128, BL], f32, tag="psh",
                                    name=f"{name}_y")
                nc.tensor.matmul(y_ps[:], par[:, w_lo:w_lo + 128], xt_sb[:],
                                 start=True, stop=True)
                y = head.tile([128, BL], f32, tag=f"{name}_relu")
                nc.vector.tensor_scalar(
                    out=y[:], in0=y_ps[:],
                    scalar1=par[:, b_col:b_col + 1], scalar2=0.0,
                    op0=mybir.AluOpType.add, op1=mybir.AluOpType.max)
                z = head.tile([128, BL], f32, tag=f"{name}_bn")
                nc.vector.tensor_scalar(
                    out=z[:], in0=y[:],
                    scalar1=par[:, bng_col:bng_col + 1],
                    scalar2=par[:, bnb_col:bnb_col + 1],
                    op0=mybir.AluOpType.mult, op1=mybir.AluOpType.add)
                return z

            h1 = layer_norm(x0_ps, 6, "ln1")
            z1 = linear_relu_bn(h1, 12, 0, 2, 3, "l1")
            z1t_ps = ps_head.tile([BL, 128], f32, tag="psh", name="z1t")
            nc.tensor.transpose(z1t_ps[:], z1[:], ident[:, :])
            h2 = layer_norm(z1t_ps, 7, "ln2")
            z2 = linear_relu_bn(h2, 140, 1, 4, 5, "l2")

            out_ps = ps_head.tile([BL, 128], f32, tag="psh", name="outT")
            nc.tensor.transpose(out_ps[:], z2[:], ident[:, :])
            out_sb = head.tile([BL, 128], f32)
            nc.vector.tensor_copy(out_sb[:], out_ps[:])
            nc.scalar.dma_start(out_d[:, :], out_sb[:])

    nc.compile()
    return nc


def _prepare_in_maps(inputs):
    import ml_dtypes

    x = np.asarray(inputs["inputs"])
    emb = np.asarray(inputs["emb"], dtype=np.float32)
    w1 = np.asarray(inputs["w1"], dtype=np.float32)
    b1 = np.asarray(inputs["b1"], dtype=np.float32)
    w2 = np.asarray(inputs["w2"], dtype=np.float32)
    b2 = np.asarray(inputs["b2"], dtype=np.float32)
    ln1_g = np.asarray(inputs["ln1_g"], np.float32)
    ln1_b = np.asarray(inputs["ln1_b"], np.float32)
    ln2_g = np.asarray(inputs["ln2_g"], np.float32)
    ln2_b = np.asarray(inputs["ln2_b"], np.float32)

    par = np.zeros((128, NPAR), dtype=np.float32)
    # y = W @ (g*xn + b) + b1 = (W*g) @ xn + (W@b + b1)
    w1f = w1 * ln1_g[None, :]
    b1f = b1 + w1 @ ln1_b
    w2f = w2 * ln2_g[None, :]
    b2f = b2 + w2 @ ln2_b
    par[:, 0] = b1f
    par[:, 1] = b2f
    par[:, 2] = np.asarray(inputs["bn1_g"], np.float32) / np.sqrt(
        np.float32(1.0) + np.float32(EPS))
    par[:, 3] = inputs["bn1_b"]
    par[:, 4] = np.asarray(inputs["bn2_g"], np.float32) / np.sqrt(
        np.float32(1.0) + np.float32(EPS))
    par[:, 5] = inputs["bn2_b"]
    par[:, 6] = EPS * P * P
    par[:, 7] = EPS
    for i in range(NB):
        par[i, 8 + i // P] = 1.0
    par[:, 12:140] = w1f.T
    par[:, 140:268] = w2f.T

    x_flat = np.asarray(x).reshape(BP, E)
    in_maps = []
    for c in range(NCORES):
        lo = c * E_SH
        seg_t = np.zeros((E_PAD, BP), dtype=np.int8)
        seg_t[:E_SH] = (x_flat[:, lo:lo + E_SH].T == 1)
        x_sh = np.ascontiguousarray(
            seg_t.reshape(NSUB, SUB, BP).transpose(1, 0, 2)
        ).reshape(SUB, NSUB * BP).astype(ml_dtypes.float8_e4m3)
        seg_e = np.zeros((E_PAD, 129), dtype=np.float32)
        seg_e[:E_SH, 0:128] = emb[lo:lo + E_SH, :]
        if c == 0:
            seg_e[0, 0:128] = 0.0   # padding_idx=0
        seg_e[:, 128] = 1.0         # count column
        emb_sh = np.ascontiguousarray(
            seg_e.reshape(NSUB, SUB, 129).transpose(1, 0, 2)
        ).reshape(SUB, NSUB * 129).astype(ml_dtypes.bfloat16)
        in_maps.append({"x": x_sh, "emb": emb_sh, "par": par})
    return in_maps


def _run(inputs, trace=False):
    from concourse.bass_utils import run_bass_kernel_spmd

    if "nc" not in _cached:
        _cached["nc"] = _build()
    nc = _cached["nc"]
    in_maps = _prepare_in_maps(inputs)
    res = run_bass_kernel_spmd(
        nc, in_maps, core_ids=list(range(NCORES)), trace=trace)
    out = np.concatenate(
        [np.asarray(res.results[c]["out"]) for c in range(NCORES)], axis=0)
    return out, res.exec_time_ns


def kernel(**inputs) -> np.ndarray:
    out, _ = _run(inputs, trace=False)
    return out
